# revision 16
# baseline (speedup 1.0000x reference)
"""Multi-LoRA routed adapter kernel for Trainium2 (8 NeuronCores).

Problem: out[b] = (x[b] @ B[aid[b]].T) @ A[aid[b]].T * (alpha/rank)
  x: [8, 1024, 2048] f32, A: [8, 2048, 16] f32, B: [8, 16, 2048] f32,
  adapter_ids: [8] i32, alpha/rank = 16/16 = 1.0.

Strategy: data-parallel over batch — sample b runs on core b. The
adapter gather (routing) is resolved host-side: each core receives only
its sample's selected A/B, pre-transposed so all device DMAs are
contiguous and the contraction dims land on SBUF partitions.

Wire formats / transport plan (body time is set by the PSUM-drain rate
and by when that stream can START, so the schedule optimizes both):
  - x pieces 0 and 2 are fp16 on the HWDGE (sync) ring: piece 0 feeds
    the very first mm1 — HWDGE arrival is reliable (~10.5 us incl. its
    ~0.6 us completion-sem, vs ~12.3-14.5 us with jitter on SWDGE) —
    and piece 2 rides along later when the ring is otherwise idle.
  - x pieces 1 and 3 are int8 (per-tensor scale dx folded into B^T),
    cast int8 -> fp16 inline by the SWDGE (gpsimd) DMA path. The SWDGE
    stream is GATED on piece-0-half-0's arrival (a 4-elem gpsimd copy
    creates the dependency) so it cannot steal SDMA/HBM bandwidth from
    the critical early HWDGE transfers (measured: ungated it slows
    piece 0 by ~2 us).
  - y is int8: 1/dy is folded into A^T host-side, so PSUM already holds
    y/dy and the PSUM->SBUF drain (ACT/DVE copy) does the
    round-to-nearest + saturate cast for free. dy is calibrated from a
    small host-side token sample (margin 1.3x; verified non-clipping).
  - AT128[p] = A^T[p mod 16] is a replicated 512 KB fp16 const loaded
    in 4 chunks on the HWDGE ring (on-device build would cost PE slots
    and a 2048-elem drain on the bottleneck engines).
  Measured end-to-end rel err ~1.4e-2 (tol 2e-2).

Per-core device kernel, 4 pieces of 256 tokens:
  mm1 (col-tiled): the PE array is split into 4 column strips via
    tile_position=(0, 32j); strip j holds BT for k-tile group j; strips
    are paired by load-half so each half can start as it lands.
  mm2: lhsT = the full [128, 128-token] Bx slab (zero holes), rhs =
    AT128 chunks; one 512-col fp32 PSUM bank per matmul.

Scheduling model (all HW-measured on this kernel):
  - o-drain floor: PSUM fp32 reads at ~1.1-1.2 ns/elem/partition and
    only DVE+ACT can read PSUM -> 16K elems/partition ~= 10.4 us. The
    schedule starts this stream as early as possible and keeps it
    dense. Slab halves alternate DVE/ACT on disjoint banks; the final
    slab drains per-512-chunk; stores are per-half there.
  - HAM clock gate: the PE runs at 1.2 GHz until it has been busy
    ~3.1-6.2 us with NO >~0.5-1 us gap (a gap resets the accumulation,
    and post-flip gaps >~1 us re-throttle). N_WARM junk matmuls bridge
    PE-start (~7.7 us) to piece-0 readiness; explicit junk bridges
    cover every later seam (piece-0 h0->h1, bx(0) drain, mm1(pc) x
    waits). Junk matmuls write a psbx-rotation PSUM region that mm1
    strip-0 later fully overwrites with start=True.
  - mm1(pc+1) sits between mm2(pc)'s two slabs: its bx drain enqueues
    ahead of slab-1's ACT o-drain, so bx is always ready before
    mm2(pc+1) and the PE never gaps on it.
"""

import os

import numpy as np

import concourse.bass as bass
import concourse.mybir as mybir
import concourse.tile as tile
from concourse import bacc
from concourse.bass_utils import run_bass_kernel_spmd

# Problem constants (hardcoded per spec).
N_CORES = 8
BATCH = 8
N_TOK = 1024
D_IN = 2048
D_OUT = 2048
RANK = 16
SCALING = 16.0 / 16.0  # alpha / rank

P = 128
K_TILES = D_IN // P  # 16
KH = K_TILES // 2  # 8 k-tiles per load chunk
KG = 4  # k-tiles per PE column strip (4 strips)
PIECE = 256  # tokens per piece
N_PIECES = N_TOK // PIECE  # 4
SLABS = PIECE // P  # 2
O_CHUNK = 512  # one fp32 PSUM bank per matmul
N_WARM = 26

# y-quant calibration: sample this many tokens per sample on the host,
# scale the observed max by this margin.
CAL_TOKENS = 64
CAL_MARGIN = 1.30

F32 = mybir.dt.float32
F16 = mybir.dt.float16
I8 = mybir.dt.int8

HW_PIECES = (0, 2)  # fp16 via HWDGE ring
SW_PIECES = (1, 3)  # int8 via gated SWDGE

_last_results = None  # stashed BassKernelResults for test harness introspection
_nc_cache = None  # compiled Bass module, reused across kernel() calls


def _build_nc() -> bass.Bass:
    nc = bacc.Bacc(None, enable_asserts=False, enable_partition_id=False)
    # Layouts: x*[h, p, kt*PIECE + j] = x[b][pc*PIECE + j, (h*KH+kt)*128+p]
    xp0 = nc.dram_tensor("xp0", [2, P, KH * PIECE], F16, kind="ExternalInput")
    xp2 = nc.dram_tensor("xp2", [2, P, KH * PIECE], F16, kind="ExternalInput")
    xq = nc.dram_tensor(
        "xq", [2, 2, P, KH * PIECE], I8, kind="ExternalInput"
    )  # pieces 1, 3
    BTp = nc.dram_tensor("BTp", [P, K_TILES * RANK], F16, kind="ExternalInput")
    AT128 = nc.dram_tensor("AT128", [P, D_OUT], F16, kind="ExternalInput")
    y = nc.dram_tensor("y", [N_TOK, D_OUT], I8, kind="ExternalOutput")

    with tile.TileContext(nc) as tc:
        with (
            tc.tile_pool(name="const", bufs=1) as cpool,
            tc.tile_pool(name="xin", bufs=2 * N_PIECES) as xpool,
            tc.tile_pool(name="bx", bufs=2) as bxpool,
            tc.tile_pool(name="outb", bufs=4) as opool,
            tc.tile_pool(name="psbx", bufs=2, space="PSUM") as psbx,
            tc.tile_pool(name="pso", bufs=3, space="PSUM") as pso,
        ):
            x_sbs = [[None, None] for _ in range(N_PIECES)]

            # HWDGE ring order: BT (64 KB, feeds mm1) -> piece-0 halves
            # -> AT128 chunks (needed from ~13.5 us) -> piece-2 halves.
            bt_sb = cpool.tile([P, K_TILES, RANK], F16)
            nc.sync.dma_start(
                bt_sb[:], BTp.rearrange("p (kt r) -> p kt r", r=RANK)
            )
            for h in range(2):
                t = xpool.tile([P, KH, PIECE], F16, tag="x")
                nc.sync.dma_start(
                    t[:], xp0[h].rearrange("p (kt n) -> p kt n", n=PIECE)
                )
                x_sbs[0][h] = t
            at_sb = cpool.tile([P, D_OUT], F16)
            for c in range(4):
                nc.sync.dma_start(
                    at_sb[:, c * O_CHUNK : (c + 1) * O_CHUNK],
                    AT128[:, c * O_CHUNK : (c + 1) * O_CHUNK],
                )
            for h in range(2):
                t = xpool.tile([P, KH, PIECE], F16, tag="x")
                nc.sync.dma_start(
                    t[:], xp2[h].rearrange("p (kt n) -> p kt n", n=PIECE)
                )
                x_sbs[2][h] = t

            # PE warm-up junk: memset is GpSimd's FIRST instruction so the
            # PE's first LDWEIGHTS can issue right after the preamble.
            junk = cpool.tile([P, P], F16)
            nc.gpsimd.memset(junk[:], 1.0)
            # SWDGE gate: this 4-elem copy depends on piece-0-half-0, so
            # the Pool queue (and with it all SWDGE x emissions) holds
            # until the critical HWDGE transfers are done.
            gate = cpool.tile([1, 4], F16)
            nc.gpsimd.tensor_copy(gate[:], x_sbs[0][0][0:1, 0, 0:4])
            for i, pc in enumerate(SW_PIECES):
                for h in range(2):
                    t = xpool.tile([P, KH, PIECE], F16, tag="x")
                    nc.gpsimd.dma_start(
                        t[:], xq[i, h].rearrange("p (kt n) -> p kt n", n=PIECE)
                    )
                    x_sbs[pc][h] = t

            # Pre-zero both PSUM bx slots: mm1's column strips write only
            # partitions 32j..32j+15; the hole partitions must stay zero
            # (they feed mm2's lhsT, nulling the replicated AT128 rows).
            for _ in range(2):
                z = psbx.tile([P, PIECE], F32, tag="ps_bx")
                nc.vector.memset(z[:], 0.0)

            # Warm tile rides the psbx rotation (buffer 0): junk/bridge
            # matmuls write partitions 0-15 cols 0-128 only — mm1 strip-0
            # later fully overwrites that region with start=True.
            ps_w = psbx.tile([P, PIECE], F32, tag="ps_bx")

            def warm(n, start):
                for w in range(n):
                    nc.tensor.matmul(
                        ps_w[:RANK, :P],
                        junk[:, :RANK],
                        junk[:],
                        start=(start and w == 0),
                        stop=False,
                        skip_group_check=True,
                    )

            warm(N_WARM, True)

            def mm1(pc):
                # 4 column strips; strip j accumulates k-tile group j
                # (kt = 4j..4j+3) into PSUM partitions 32j..32j+15,
                # paired by load-half so each half starts as it lands.
                ps_bx = psbx.tile([P, PIECE], F32, tag="ps_bx")
                for h in range(2):
                    if h == 1 and pc == 0:
                        # Bridge the piece-0 h0 -> h1 arrival stagger.
                        warm(4, False)
                    for k in range(KG):
                        for j in (2 * h, 2 * h + 1):
                            kt = j * KG + k
                            kh = kt - h * KH
                            nc.tensor.matmul(
                                ps_bx[32 * j : 32 * j + RANK, :],
                                bt_sb[:, kt, :],
                                x_sbs[pc][h][:, kh, :],
                                start=(k == 0),
                                stop=(k == KG - 1),
                                tile_position=(0, 32 * j),
                                skip_group_check=True,
                            )
                bx_sb = bxpool.tile([P, PIECE], F16)
                # bx drain split across both engines (0.24 us each) —
                # enqueued ahead of the next slab's o-drains so bx is
                # always ready before mm2 needs it.
                nc.vector.tensor_copy(bx_sb[:, : PIECE // 2], ps_bx[:, : PIECE // 2])
                nc.scalar.copy(bx_sb[:, PIECE // 2 :], ps_bx[:, PIECE // 2 :])
                return bx_sb

            def mm2_slab(bx_sb, pc, s):
                final = pc == N_PIECES - 1 and s == SLABS - 1
                o_sb = opool.tile([P, D_OUT], I8, tag="o")
                row0 = pc * PIECE + s * P
                for half in range(2):
                    ps_o = pso.tile([P, 2, O_CHUNK], F32)
                    for q in range(2):
                        oc = 2 * half + q
                        nc.tensor.matmul(
                            ps_o[:, q, :],
                            bx_sb[:, s * P : (s + 1) * P],
                            at_sb[:, oc * O_CHUNK : (oc + 1) * O_CHUNK],
                            start=True,
                            stop=True,
                        )
                    # Drain split: DVE half 0, ACT half 1 (disjoint PSUM
                    # banks, parallel on TRN2); fp32 -> int8 RNE+saturate.
                    if not final:
                        dst = o_sb[
                            :, 2 * half * O_CHUNK : 2 * (half + 1) * O_CHUNK
                        ]
                        if half == 0:
                            nc.vector.tensor_copy(dst, ps_o[:, :, :])
                        else:
                            nc.scalar.copy(dst, ps_o[:, :, :])
                    else:
                        # Final slab: per-512-chunk drains alternating
                        # engines (tail = one chunk drain), per-half
                        # 128 KB stores.
                        for q in range(2):
                            oc = 2 * half + q
                            dst = o_sb[:, oc * O_CHUNK : (oc + 1) * O_CHUNK]
                            if (half + q) % 2 == 0:
                                nc.vector.tensor_copy(dst, ps_o[:, q, :])
                            else:
                                nc.scalar.copy(dst, ps_o[:, q, :])
                        nc.sync.dma_start(
                            y[
                                row0 : row0 + P,
                                2 * half * O_CHUNK : 2 * (half + 1) * O_CHUNK,
                            ],
                            o_sb[:, 2 * half * O_CHUNK : 2 * (half + 1) * O_CHUNK],
                        )
                if not final:
                    # Slab-granular store (256 KB int8).
                    nc.sync.dma_start(y[row0 : row0 + P, :], o_sb[:])

            # Pipeline: mm1(pc+1) sits between mm2(pc)'s slabs; small junk
            # bridges absorb the bx(0)-drain wait and x-arrival jitter so
            # the HAM activity window never resets.
            bxs = [mm1(0)]
            warm(5, False)
            # NOTE: no junk bridges inside this loop — ps_w aliases the
            # psbx buffer that holds bx(odd pieces), and a bridge matmul
            # could race that tile's in-flight DVE half-drain (cols
            # 0-127 overlap). The pre-loop bridges write before any bx
            # drain exists and mm1 strip-0 overwrites with start=True.
            for pc in range(N_PIECES):
                mm2_slab(bxs[pc], pc, 0)
                if pc + 1 < N_PIECES:
                    bxs.append(mm1(pc + 1))
                mm2_slab(bxs[pc], pc, 1)
    nc.compile()
    return nc


def kernel(x, A, B, adapter_ids):
    global _last_results
    x = np.asarray(x, dtype=np.float32)
    A = np.asarray(A, dtype=np.float32)
    B = np.asarray(B, dtype=np.float32)
    adapter_ids = np.asarray(adapter_ids)

    assert x.shape == (BATCH, N_TOK, D_IN)

    # Per-tensor x quantization scale (exact, host-side).
    dx = np.float32(np.abs(x).max() / 127.0)
    # y scale: calibrate on a token sample per batch, with margin.
    ymax = 0.0
    for b in range(BATCH):
        aid = int(adapter_ids[b])
        xs = x[b, :: N_TOK // CAL_TOKENS]
        ys = (xs @ B[aid].T) @ (A[aid].T * np.float32(SCALING))
        ymax = max(ymax, float(np.abs(ys).max()))
    dy = np.float32(ymax * CAL_MARGIN / 127.0)

    def pack(xpiece):
        # [PIECE, 2, KH, P] -> [2, P, KH*PIECE] rows contiguous per (h, p).
        return np.ascontiguousarray(
            xpiece.reshape(PIECE, 2, KH, P)
            .transpose(1, 3, 2, 0)
            .reshape(2, P, KH * PIECE)
        )

    in_maps = []
    for b in range(BATCH):
        aid = int(adapter_ids[b])
        # Fold the LoRA scaling and 1/dy into A; replicate to 128
        # partitions (AT128[p] = A^T[p % 16]).
        At = (A[aid].T * np.float32(SCALING / dy)).astype(np.float16)
        At128 = np.ascontiguousarray(np.tile(At, (P // RANK, 1)))
        # Fold dx into B. Pack B^T to [p, kt*r].
        BTp = np.ascontiguousarray(
            (B[aid].T * dx)
            .reshape(K_TILES, P, RANK)
            .transpose(1, 0, 2)
            .reshape(P, K_TILES * RANK)
            .astype(np.float16)
        )
        xs = x[b] / dx  # common 1/dx scale (BT carries dx)
        xp0 = pack(xs[0 * PIECE : 1 * PIECE].astype(np.float16))
        xp2 = pack(xs[2 * PIECE : 3 * PIECE].astype(np.float16))
        xq = np.stack(
            [
                pack(
                    np.clip(np.rint(xs[pc * PIECE : (pc + 1) * PIECE]), -127, 127
                    ).astype(np.int8)
                )
                for pc in SW_PIECES
            ]
        )
        in_maps.append(
            {"xp0": xp0, "xp2": xp2, "xq": xq, "BTp": BTp, "AT128": At128}
        )

    global _nc_cache
    if _nc_cache is None:
        _nc_cache = _build_nc()
    nc = _nc_cache
    trace = bool(int(os.environ.get("KERNEL_BASS_TRACE", "0")))
    res = run_bass_kernel_spmd(
        nc, in_maps, core_ids=list(range(N_CORES)), trace=trace
    )
    _last_results = res

    out = np.empty((BATCH, N_TOK, D_OUT), dtype=np.float32)
    for b in range(BATCH):
        out[b] = res.results[b]["y"].astype(np.float32) * dy
    return out


# revision 19
# speedup vs baseline: 1.0856x; 1.0856x over previous
"""Multi-LoRA routed adapter kernel for Trainium2 (8 NeuronCores).

Problem: out[b] = (x[b] @ B[aid[b]].T) @ A[aid[b]].T * (alpha/rank)
  x: [8, 1024, 2048] f32, A: [8, 2048, 16] f32, B: [8, 16, 2048] f32,
  adapter_ids: [8] i32, alpha/rank = 16/16 = 1.0.

Strategy: data-parallel over batch — sample b runs on core b. The
adapter gather (routing) is resolved host-side: each core receives only
its sample's selected A/B, pre-transposed so all device DMAs are
contiguous and the contraction dims land on SBUF partitions.

Wire formats / transport plan (body time is set by the PSUM-drain rate
and by when that stream can START, so the schedule optimizes both):
  - x pieces 0 and 2 are fp16 on the HWDGE (sync) ring: piece 0 feeds
    the very first mm1 — HWDGE arrival is reliable (~10.5 us incl. its
    ~0.6 us completion-sem, vs ~12.3-14.5 us with jitter on SWDGE) —
    and piece 2 rides along later when the ring is otherwise idle.
  - x pieces 1 and 3 are int8 (per-tensor scale dx folded into B^T),
    cast int8 -> fp16 inline by the SWDGE (gpsimd) DMA path. The SWDGE
    stream is GATED on piece-0-half-0's arrival (a 4-elem gpsimd copy
    creates the dependency) so it cannot steal SDMA/HBM bandwidth from
    the critical early HWDGE transfers (measured: ungated it slows
    piece 0 by ~2 us).
  - y is int8: 1/dy is folded into A^T host-side, so PSUM already holds
    y/dy and the PSUM->SBUF drain (ACT/DVE copy) does the
    round-to-nearest + saturate cast for free. dy is calibrated from a
    small host-side token sample (margin 1.3x; verified non-clipping).
  - AT128[p] = A^T[p mod 16] is a replicated 512 KB fp16 const loaded
    in 4 chunks on the HWDGE ring (on-device build would cost PE slots
    and a 2048-elem drain on the bottleneck engines).
  Measured end-to-end rel err ~1.4e-2 (tol 2e-2).

Per-core device kernel, 4 pieces of 256 tokens:
  mm1 (col-tiled): the PE array is split into 4 column strips via
    tile_position=(0, 32j); strip j holds BT for k-tile group j; strips
    are paired by load-half so each half can start as it lands.
  mm2: lhsT = the full [128, 128-token] Bx slab (zero holes), rhs =
    AT128 chunks; one 512-col fp32 PSUM bank per matmul.

Scheduling model (all HW-measured on this kernel):
  - o-drain floor: PSUM fp32 reads at ~1.1-1.2 ns/elem/partition and
    only DVE+ACT can read PSUM -> 16K elems/partition ~= 10.4 us. The
    schedule starts this stream as early as possible and keeps it
    dense. Slab halves alternate DVE/ACT on disjoint banks; the final
    slab drains per-512-chunk; stores are per-half there.
  - HAM clock gate: the PE runs at 1.2 GHz until it has been busy
    ~3.1-6.2 us with NO >~0.5-1 us gap (a gap resets the accumulation,
    and post-flip gaps >~1 us re-throttle). N_WARM junk matmuls bridge
    PE-start (~7.7 us) to piece-0 readiness; explicit junk bridges
    cover every later seam (piece-0 h0->h1, bx(0) drain, mm1(pc) x
    waits). Junk matmuls write a psbx-rotation PSUM region that mm1
    strip-0 later fully overwrites with start=True.
  - mm1(pc+1) sits between mm2(pc)'s two slabs: its bx drain enqueues
    ahead of slab-1's ACT o-drain, so bx is always ready before
    mm2(pc+1) and the PE never gaps on it.
"""

import os

import numpy as np

import concourse.bass as bass
import concourse.mybir as mybir
import concourse.tile as tile
from concourse import bacc
from concourse.bass_utils import run_bass_kernel_spmd

# Problem constants (hardcoded per spec).
N_CORES = 8
BATCH = 8
N_TOK = 1024
D_IN = 2048
D_OUT = 2048
RANK = 16
SCALING = 16.0 / 16.0  # alpha / rank

P = 128
K_TILES = D_IN // P  # 16
KH = K_TILES // 2  # 8 k-tiles per load chunk
KG = 4  # k-tiles per PE column strip (4 strips)
PIECE = 256  # tokens per piece
N_PIECES = N_TOK // PIECE  # 4
SLABS = PIECE // P  # 2
O_CHUNK = 512  # one fp32 PSUM bank per matmul
N_WARM = 32

# y-quant calibration: sample this many tokens per sample on the host,
# scale the observed max by this margin.
CAL_TOKENS = 64
CAL_MARGIN = 1.30

F32 = mybir.dt.float32
F16 = mybir.dt.float16
I8 = mybir.dt.int8

HW_PIECES = (0,)  # fp16 via HWDGE ring
SW_PIECES = (1, 2, 3)  # int8 via delayed SWDGE

_last_results = None  # stashed BassKernelResults for test harness introspection
_nc_cache = None  # compiled Bass module, reused across kernel() calls


def _build_nc() -> bass.Bass:
    nc = bacc.Bacc(None, enable_asserts=False, enable_partition_id=False)
    # Layouts: x*[h, p, kt*PIECE + j] = x[b][pc*PIECE + j, (h*KH+kt)*128+p]
    xp0 = nc.dram_tensor("xp0", [2, P, KH * PIECE], F16, kind="ExternalInput")
    xq = nc.dram_tensor(
        "xq", [3, 2, P, KH * PIECE], I8, kind="ExternalInput"
    )  # pieces 1, 2, 3
    BTp = nc.dram_tensor("BTp", [P, K_TILES * RANK], F16, kind="ExternalInput")
    AT128 = nc.dram_tensor("AT128", [P, D_OUT], F16, kind="ExternalInput")
    y = nc.dram_tensor("y", [N_TOK, D_OUT], I8, kind="ExternalOutput")

    with tile.TileContext(nc) as tc:
        with (
            tc.tile_pool(name="const", bufs=1) as cpool,
            tc.tile_pool(name="xin", bufs=2 * N_PIECES) as xpool,
            tc.tile_pool(name="bx", bufs=2) as bxpool,
            tc.tile_pool(name="outb", bufs=4) as opool,
            tc.tile_pool(name="psbx", bufs=2, space="PSUM") as psbx,
            tc.tile_pool(name="pso", bufs=3, space="PSUM") as pso,
        ):
            x_sbs = [[None, None] for _ in range(N_PIECES)]

            # HWDGE ring order: BT (64 KB, feeds mm1) -> x0h0 -> AT128
            # chunk 0 (feeds mm2(0)'s first matmul) -> x0h1 -> AT128
            # chunks 1-3.
            bt_sb = cpool.tile([P, K_TILES, RANK], F16)
            nc.sync.dma_start(
                bt_sb[:], BTp.rearrange("p (kt r) -> p kt r", r=RANK)
            )
            at_sb = cpool.tile([P, D_OUT], F16)
            for h in range(2):
                t = xpool.tile([P, KH, PIECE], F16, tag="x")
                nc.sync.dma_start(
                    t[:], xp0[h].rearrange("p (kt n) -> p kt n", n=PIECE)
                )
                x_sbs[0][h] = t
                nc.sync.dma_start(
                    at_sb[:, h * O_CHUNK : (h + 1) * O_CHUNK],
                    AT128[:, h * O_CHUNK : (h + 1) * O_CHUNK],
                )
            for c in range(2, 4):
                nc.sync.dma_start(
                    at_sb[:, c * O_CHUNK : (c + 1) * O_CHUNK],
                    AT128[:, c * O_CHUNK : (c + 1) * O_CHUNK],
                )

            # PE warm-up junk: memset is GpSimd's FIRST instruction so the
            # PE's first LDWEIGHTS can issue right after the preamble.
            junk = cpool.tile([P, P], F16)
            nc.gpsimd.memset(junk[:], 1.0)
            # Timed SWDGE delay (~2.6 us of Pool memsets): holds the SWDGE
            # x emissions until the critical HWDGE x0 transfer is past its
            # bandwidth window. A data-dependency gate would be cleaner
            # but gpsimd SBUF ops have multi-us dispatch cost (measured).
            dummy = cpool.tile([P, 2048], F16)
            nc.gpsimd.memset(dummy[:], 0.0)
            nc.gpsimd.memset(dummy[:, :1024], 1.0)
            for i, pc in enumerate(SW_PIECES):
                for h in range(2):
                    t = xpool.tile([P, KH, PIECE], F16, tag="x")
                    nc.gpsimd.dma_start(
                        t[:], xq[i, h].rearrange("p (kt n) -> p kt n", n=PIECE)
                    )
                    x_sbs[pc][h] = t

            # Pre-zero both PSUM bx slots: mm1's column strips write only
            # partitions 32j..32j+15; the hole partitions must stay zero
            # (they feed mm2's lhsT, nulling the replicated AT128 rows).
            for _ in range(2):
                z = psbx.tile([P, PIECE], F32, tag="ps_bx")
                nc.vector.memset(z[:], 0.0)

            # Warm tile rides the psbx rotation (buffer 0): junk/bridge
            # matmuls write partitions 0-15 cols 0-128 only — mm1 strip-0
            # later fully overwrites that region with start=True.
            ps_w = psbx.tile([P, PIECE], F32, tag="ps_bx")

            def warm(n, start):
                for w in range(n):
                    nc.tensor.matmul(
                        ps_w[:RANK, :P],
                        junk[:, :RANK],
                        junk[:],
                        start=(start and w == 0),
                        stop=False,
                        skip_group_check=True,
                    )

            warm(N_WARM, True)

            def mm1(pc):
                # 4 column strips; strip j accumulates k-tile group j
                # (kt = 4j..4j+3) into PSUM partitions 32j..32j+15,
                # paired by load-half so each half starts as it lands.
                ps_bx = psbx.tile([P, PIECE], F32, tag="ps_bx")
                for h in range(2):
                    if h == 1 and pc == 0:
                        # Bridge the piece-0 h0 -> h1 arrival stagger.
                        warm(4, False)
                    for k in range(KG):
                        for j in (2 * h, 2 * h + 1):
                            kt = j * KG + k
                            kh = kt - h * KH
                            nc.tensor.matmul(
                                ps_bx[32 * j : 32 * j + RANK, :],
                                bt_sb[:, kt, :],
                                x_sbs[pc][h][:, kh, :],
                                start=(k == 0),
                                stop=(k == KG - 1),
                                tile_position=(0, 32 * j),
                                skip_group_check=True,
                            )
                bx_sb = bxpool.tile([P, PIECE], F16)
                # bx drain split across both engines (0.24 us each) —
                # enqueued ahead of the next slab's o-drains so bx is
                # always ready before mm2 needs it.
                nc.vector.tensor_copy(bx_sb[:, : PIECE // 2], ps_bx[:, : PIECE // 2])
                nc.scalar.copy(bx_sb[:, PIECE // 2 :], ps_bx[:, PIECE // 2 :])
                return bx_sb

            def mm2_slab(bx_sb, pc, s):
                final = pc == N_PIECES - 1 and s == SLABS - 1
                o_sb = opool.tile([P, D_OUT], I8, tag="o")
                row0 = pc * PIECE + s * P
                for half in range(2):
                    ps_o = pso.tile([P, 2, O_CHUNK], F32)
                    for q in range(2):
                        oc = 2 * half + q
                        nc.tensor.matmul(
                            ps_o[:, q, :],
                            bx_sb[:, s * P : (s + 1) * P],
                            at_sb[:, oc * O_CHUNK : (oc + 1) * O_CHUNK],
                            start=True,
                            stop=True,
                        )
                    # Drain split: DVE half 0, ACT half 1 (disjoint PSUM
                    # banks, parallel on TRN2); fp32 -> int8 RNE+saturate.
                    if not final:
                        dst = o_sb[
                            :, 2 * half * O_CHUNK : 2 * (half + 1) * O_CHUNK
                        ]
                        if half == 0:
                            nc.vector.tensor_copy(dst, ps_o[:, :, :])
                        else:
                            nc.scalar.copy(dst, ps_o[:, :, :])
                    else:
                        # Final slab: per-512-chunk drains alternating
                        # engines (tail = one chunk drain), per-half
                        # 128 KB stores.
                        for q in range(2):
                            oc = 2 * half + q
                            dst = o_sb[:, oc * O_CHUNK : (oc + 1) * O_CHUNK]
                            if (half + q) % 2 == 0:
                                nc.vector.tensor_copy(dst, ps_o[:, q, :])
                            else:
                                nc.scalar.copy(dst, ps_o[:, q, :])
                        nc.sync.dma_start(
                            y[
                                row0 : row0 + P,
                                2 * half * O_CHUNK : 2 * (half + 1) * O_CHUNK,
                            ],
                            o_sb[:, 2 * half * O_CHUNK : 2 * (half + 1) * O_CHUNK],
                        )
                if not final:
                    # Slab-granular store (256 KB int8).
                    nc.sync.dma_start(y[row0 : row0 + P, :], o_sb[:])

            # Pipeline: mm1(pc+1) sits between mm2(pc)'s slabs; small junk
            # bridges absorb the bx(0)-drain wait and x-arrival jitter so
            # the HAM activity window never resets.
            bxs = [mm1(0)]
            warm(5, False)
            # NOTE: no junk bridges inside this loop — ps_w aliases the
            # psbx buffer that holds bx(odd pieces), and a bridge matmul
            # could race that tile's in-flight DVE half-drain (cols
            # 0-127 overlap). The pre-loop bridges write before any bx
            # drain exists and mm1 strip-0 overwrites with start=True.
            for pc in range(N_PIECES):
                mm2_slab(bxs[pc], pc, 0)
                if pc + 1 < N_PIECES:
                    bxs.append(mm1(pc + 1))
                mm2_slab(bxs[pc], pc, 1)
    nc.compile()
    return nc


def kernel(x, A, B, adapter_ids):
    global _last_results
    x = np.asarray(x, dtype=np.float32)
    A = np.asarray(A, dtype=np.float32)
    B = np.asarray(B, dtype=np.float32)
    adapter_ids = np.asarray(adapter_ids)

    assert x.shape == (BATCH, N_TOK, D_IN)

    # Per-tensor x quantization scale (exact, host-side).
    dx = np.float32(np.abs(x).max() / 127.0)
    # y scale: calibrate on a token sample per batch, with margin.
    ymax = 0.0
    for b in range(BATCH):
        aid = int(adapter_ids[b])
        xs = x[b, :: N_TOK // CAL_TOKENS]
        ys = (xs @ B[aid].T) @ (A[aid].T * np.float32(SCALING))
        ymax = max(ymax, float(np.abs(ys).max()))
    dy = np.float32(ymax * CAL_MARGIN / 127.0)

    def pack(xpiece):
        # [PIECE, 2, KH, P] -> [2, P, KH*PIECE] rows contiguous per (h, p).
        return np.ascontiguousarray(
            xpiece.reshape(PIECE, 2, KH, P)
            .transpose(1, 3, 2, 0)
            .reshape(2, P, KH * PIECE)
        )

    in_maps = []
    for b in range(BATCH):
        aid = int(adapter_ids[b])
        # Fold the LoRA scaling and 1/dy into A; replicate to 128
        # partitions (AT128[p] = A^T[p % 16]).
        At = (A[aid].T * np.float32(SCALING / dy)).astype(np.float16)
        At128 = np.ascontiguousarray(np.tile(At, (P // RANK, 1)))
        # Fold dx into B. Pack B^T to [p, kt*r].
        BTp = np.ascontiguousarray(
            (B[aid].T * dx)
            .reshape(K_TILES, P, RANK)
            .transpose(1, 0, 2)
            .reshape(P, K_TILES * RANK)
            .astype(np.float16)
        )
        xs = x[b] / dx  # common 1/dx scale (BT carries dx)
        xp0 = pack(xs[0 * PIECE : 1 * PIECE].astype(np.float16))
        xq = np.stack(
            [
                pack(
                    np.clip(np.rint(xs[pc * PIECE : (pc + 1) * PIECE]), -127, 127
                    ).astype(np.int8)
                )
                for pc in SW_PIECES
            ]
        )
        in_maps.append({"xp0": xp0, "xq": xq, "BTp": BTp, "AT128": At128})

    global _nc_cache
    if _nc_cache is None:
        _nc_cache = _build_nc()
    nc = _nc_cache
    trace = bool(int(os.environ.get("KERNEL_BASS_TRACE", "0")))
    res = run_bass_kernel_spmd(
        nc, in_maps, core_ids=list(range(N_CORES)), trace=trace
    )
    _last_results = res

    out = np.empty((BATCH, N_TOK, D_OUT), dtype=np.float32)
    for b in range(BATCH):
        out[b] = res.results[b]["y"].astype(np.float32) * dy
    return out
